# revision 34
# baseline (speedup 1.0000x reference)
"""RNN-T decoder + joint network Trainium2 kernel (8-core SPMD).

Sharding: data-parallel over batch B=8 -> one batch element per core.
Each core runs, fully on-device:
  2-layer LSTM over U=50 steps (weight-stationary fp16 matmuls on PE,
  gate activations on ScalarE, cell updates on VectorE/GpSimdE).  The
  recurrence is bound by the per-matmul weight-switch cost (~33ns x 64
  matmuls per layer-step), so everything else is arranged to hide under
  it: x-parts of the gate pre-activations are batched matmuls computed
  in XB-step blocks (W_ih0 @ eys blocks fill the pipeline ramp; W_ih1 @
  h0 blocks run just-in-time with layer 1 lagging LAG=XB+1 slots, each
  block accumulating in one PSUM tile and evicted by a single in-place
  DVE add so evictions never sit on the h-chain).  Gate matmuls run at
  16x scale (weights/biases staged x16; the activation scale port
  undoes it) -- numerically exact, kept from an fp8 experiment.
  z_enc = W_enc @ hs^T + b_enc is scheduled into the ramp; z_dec =
  W_dec @ h1, the joint tanh and the first joint row-tiles are
  interleaved into the layer-1 drain (with PE backlog primed ahead of
  it) so the PE never idles between the recurrence and the joint:
  joint: tanh(z_enc + z_dec[u]) per u            (ScalarE, bias port)
         (T*U, 512) @ W_out.T -> (T*U, 1000)     (PE, fp16, fp32 accum)
         + b_out, evict                          (VectorE)
  contiguous DMA of (T*U, 1000) f32 to HBM
Weights are staged host-side as single [128, 4*width] images so each
tensor is one large DMA, issued in dependency-criticality order.
Host only stages layouts (transpose/cast/embedding-row gather) and
reassembles the u-major per-core outputs into (B, T, U, ODIM).
"""

import numpy as np

B, T, U = 8, 200, 50
E = 512          # EPROJS == DUNITS == EMB == JOINT
ODIM = 1000
BLANK = 0
NG = 16          # gate-dim tiles of 128 (4*DUNITS / 128)
KT = 4           # contraction tiles of 128 (E / 128)
NT = T * U       # joint rows, u-major: row = u*T + t
VJ = 500         # vocab split per PSUM bank
XB = 5           # xg1 block width (decoder steps per W_ih1 matmul batch)
LAG = 6          # layer-1 step t-LAG runs at slot t (xg1 blocks of XB
                 # finish a full slot before their first consumer)

_CACHE = {}


def _install_tile_patch():
    """This walrus build rejects >1 sync wait on one instruction; spread the
    Tile epilogue drain's waits across single-wait NoOp carriers."""
    import concourse.mybir as mybir
    import concourse.tile as tile_mod
    from concourse.vector_clock import ScopedClock

    if getattr(tile_mod.TileContext, "_drain_patched", False):
        return

    def _drain_and_barrier(self, tick_clock, wait_clock):
        drain_inst = self.nc.sync.drain()
        wait_clock.add_sem_waits(
            drain_inst.ins, ScopedClock({None: tick_clock.global_clock})
        )
        si = drain_inst.ins.sync_info
        if si is not None and si.on_wait and len(si.on_wait) > 1:
            waits = list(si.on_wait)
            ups = list(si.on_update) if si.on_update else []
            drain_inst.ins.sync_info = mybir.SyncInfo(
                on_wait=waits[:1], on_update=ups
            )
            for w in waits[1:]:
                nop = self.nc.sync.nop()
                nop.ins.sync_info = mybir.SyncInfo(on_wait=[w], on_update=[])
        self.nc.all_engine_barrier()
        assert self.sems is not None
        popped = self.nc._tile_sem_poison_stack.pop()
        assert popped is self._sem_poison
        self.nc.clear_and_free_semaphores(list(self.sems.allocated().values()))
        self.nc.all_engine_barrier()

    tile_mod.TileContext._drain_and_barrier = _drain_and_barrier
    tile_mod.TileContext._drain_patched = True


def _split_multi_waits(nc):
    """This walrus build allows one sync wait per instruction. Hoist excess
    waits onto single-wait NoOp carriers directly before the instruction on
    the same engine (program order on the sequencer preserves semantics)."""
    import concourse.mybir as mybir

    n_new = 0
    for fn in nc.m.functions:
        for blk in fn.blocks:
            ins = blk.instructions
            out = []
            dirty = False
            for inst in ins:
                si = inst.sync_info
                if si is not None and si.on_wait and len(si.on_wait) > 1:
                    waits = list(si.on_wait)
                    ups = list(si.on_update) if si.on_update else []
                    for w in waits[:-1]:
                        nop = mybir.InstNoOp(
                            name=f"{inst.name}_w{n_new}", ins=[], outs=[]
                        )
                        n_new += 1
                        nop.engine = inst.engine
                        nop.sync_info = mybir.SyncInfo(on_wait=[w], on_update=[])
                        out.append(nop)
                    inst.sync_info = mybir.SyncInfo(
                        on_wait=[waits[-1]], on_update=ups
                    )
                    dirty = True
                out.append(inst)
            if dirty:
                blk.instructions = out
    return n_new


def _build_nc():
    import concourse.bass as bass
    import concourse.mybir as mybir
    import concourse.tile as tile

    _install_tile_patch()
    f16, f32 = mybir.dt.float16, mybir.dt.float32
    Sig = mybir.ActivationFunctionType.Sigmoid
    Tanh = mybir.ActivationFunctionType.Tanh

    nc = bass.Bass()
    # matrices staged as [128, KT*width]: chunk k = columns [k*w, (k+1)*w)
    d_eysT = nc.dram_tensor("eysT", [128, KT * U], f16, kind="ExternalInput")
    d_wih0a = nc.dram_tensor("wih0aT", [128, KT * 2 * E], f16, kind="ExternalInput")
    d_wih0b = nc.dram_tensor("wih0bT", [128, KT * 2 * E], f16, kind="ExternalInput")
    d_bg0 = nc.dram_tensor("bg0", [128, NG], f32, kind="ExternalInput")
    d_whh0 = nc.dram_tensor("whh0T", [128, KT * 4 * E], f16, kind="ExternalInput")
    d_whh1 = nc.dram_tensor("whh1T", [128, KT * 4 * E], f16, kind="ExternalInput")
    d_wih1 = nc.dram_tensor("wih1T", [128, KT * 4 * E], f16, kind="ExternalInput")
    d_bg1 = nc.dram_tensor("bg1", [128, NG], f32, kind="ExternalInput")
    d_hsT = nc.dram_tensor("hsT", [128, KT * T], f16, kind="ExternalInput")
    d_wenc = nc.dram_tensor("wencT", [128, KT * E], f16, kind="ExternalInput")
    d_benc = nc.dram_tensor("bencT", [128, KT], f32, kind="ExternalInput")
    d_wdec = nc.dram_tensor("wdecT", [128, KT * E], f16, kind="ExternalInput")
    d_wout = nc.dram_tensor("woutT", [128, KT * ODIM], f16, kind="ExternalInput")
    d_bout = nc.dram_tensor("boutB", [128, ODIM], f32, kind="ExternalInput")
    d_out = nc.dram_tensor("out", [NT, ODIM], f32, kind="ExternalOutput")

    with tile.TileContext(nc) as tc:
        with (
            tc.tile_pool(name="wp", bufs=1) as wp,
            tc.tile_pool(name="sp", bufs=1) as sp,
            tc.tile_pool(name="gp", bufs=3) as gp,
            tc.tile_pool(name="op", bufs=3) as op,
            tc.tile_pool(name="pp", bufs=1, space="PSUM") as pp,
        ):
            # ---- weight tiles, one DMA each, criticality order ----------
            def load(dram, width, dt=f16, name=""):
                t = wp.tile([128, width], dt, tag=name, name=name)
                nc.sync.dma_start(t[:], dram[:])
                return t

            wih0a = load(d_wih0a, KT * 2 * E, name="wih0a")
            eysT = load(d_eysT, KT * U, name="eysT")
            bg0 = load(d_bg0, NG, f32, name="bg0")
            bg1 = load(d_bg1, NG, f32, name="bg1")
            wih0b = load(d_wih0b, KT * 2 * E, name="wih0b")
            whh0 = load(d_whh0, KT * 4 * E, name="whh0")
            whh1 = load(d_whh1, KT * 4 * E, name="whh1")
            wih1 = load(d_wih1, KT * 4 * E, name="wih1")
            hsT = load(d_hsT, KT * T, name="hsT")
            wenc = load(d_wenc, KT * E, name="wenc")
            benc = load(d_benc, KT, f32, name="benc")
            wdec = load(d_wdec, KT * E, name="wdec")
            wout = load(d_wout, KT * ODIM, name="wout")
            bout = load(d_bout, ODIM, f32, name="bout")

            # k-chunk views
            eysTv = eysT.rearrange("p (k w) -> p k w", k=KT)
            wih0av = wih0a.rearrange("p (k w) -> p k w", k=KT)
            wih0bv = wih0b.rearrange("p (k w) -> p k w", k=KT)
            whh0v = whh0.rearrange("p (k w) -> p k w", k=KT)
            whh1v = whh1.rearrange("p (k w) -> p k w", k=KT)
            wih1v = wih1.rearrange("p (k w) -> p k w", k=KT)
            hsTv = hsT.rearrange("p (k w) -> p k w", k=KT)
            wencv = wenc.rearrange("p (k w) -> p k w", k=KT)
            wdecv = wdec.rearrange("p (k w) -> p k w", k=KT)
            woutv = wout.rearrange("p (k w) -> p k w", k=KT)

            # ---- state tiles --------------------------------------------
            zenc = [sp.tile([128, T], f32, tag=f"zenc{k}", name=f"zenc{k}") for k in range(KT)]
            zdec = [sp.tile([128, U], f32, tag=f"zdec{k}", name=f"zdec{k}") for k in range(KT)]
            tmpT = [sp.tile([128, NT], f16, tag=f"tmpT{k}", name=f"tmpT{k}") for k in range(KT)]
            xg0 = sp.tile([128, NG * U], f32, tag="xg0", name="xg0")   # [mt*U + u]
            xg1 = sp.tile([128, NG * U], f32, tag="xg1", name="xg1")
            h0h = sp.tile([128, U * KT], f16, tag="h0h", name="h0h")   # [u*KT + c]
            h1h = sp.tile([128, U * KT], f16, tag="h1h", name="h1h")
            c0 = sp.tile([128, KT], f32, tag="c0", name="c0")
            c1 = sp.tile([128, KT], f32, tag="c1", name="c1")
            nc.vector.memset(c0[:], 0.0)
            nc.vector.memset(c1[:], 0.0)

            xg0r = xg0.rearrange("p (m u) -> p m u", u=U)
            xg1r = xg1.rearrange("p (m u) -> p m u", u=U)
            h0r = h0h.rearrange("p (u c) -> p u c", c=KT)
            h1r = h1h.rearrange("p (u c) -> p u c", c=KT)

            scope = nc.named_scope

            # engine stream-order recorder (vector/scalar/gpsimd): forces
            # the in-order sequencers to see ops in emission order so the
            # two layers' chains + drain interleave don't invert.
            lstm_ord = {"vector": [], "scalar": [], "gpsimd": []}
            rec_on = [True]

            def rec(eng, bi):
                if rec_on[0]:
                    lstm_ord[eng].append(bi)
                return bi

            # ---- xg bias prefill: blocks then ACCUMULATE in place with a
            # ---- single wide DVE add (the per-mt bias evicts were ~3.7us
            # ---- of instruction overhead head-of-line blocking the chains)
            nc.vector.memset(xg0[:], 0.0)
            nc.vector.memset(xg1[:], 0.0)
            for mt in range(NG):
                nc.vector.tensor_scalar_add(
                    xg0r[:, mt, :], xg0r[:, mt, :], bg0[:, mt:mt + 1])
                nc.vector.tensor_scalar_add(
                    xg1r[:, mt, :], xg1r[:, mt, :], bg1[:, mt:mt + 1])

            # ---- building blocks ----------------------------------------
            def xg_block(wsl, rhsv_col, xgr, lo, hi, mt_lo=0, mt_hi=NG):
                # xg[:, mt, lo:hi] += W_ih^T chunk products.  All mt
                # sub-blocks accumulate into ONE psum tile; a single 3D
                # in-place add evicts the whole block.
                w = hi - lo
                n = mt_hi - mt_lo
                ps = pp.tile([128, VJ], f32, tag="js", bufs=4, name="js")
                for j, mt in enumerate(range(mt_lo, mt_hi)):
                    for k in range(KT):
                        nc.tensor.matmul(
                            ps[:, j * w:j * w + w], wsl(k, mt),
                            rhsv_col(k, lo, hi), start=(k == 0),
                            stop=(k == KT - 1),
                        )
                psv = ps[:, :n * w].rearrange("p (m w) -> p m w", w=w)
                rec("vector", nc.vector.tensor_add(
                    xgr[:, mt_lo:mt_hi, lo:hi], psv,
                    xgr[:, mt_lo:mt_hi, lo:hi]))

            def wih0sl(k, mt):
                wv = wih0av if mt < 8 else wih0bv
                m = mt % 8
                return wv[:, k, m * 128:(m + 1) * 128]

            def xg0_block(lo, hi, mt_lo=0, mt_hi=NG):
                xg_block(wih0sl, lambda k, a, b: eysTv[:, k, a:b], xg0r,
                         lo, hi, mt_lo, mt_hi)

            def xg1_block(lo, hi):
                xg_block(lambda k, mt: wih1v[:, k, mt * 128:(mt + 1) * 128],
                         lambda k, a, b: h0r[:, a:b, k], xg1r,
                         lo, hi)

            zenc_ps = []

            def zenc_part(mts):
                ps = pp.tile([128, VJ], f32, tag="js", bufs=4, name="js")
                for j, mt in enumerate(mts):
                    for k in range(KT):
                        nc.tensor.matmul(
                            ps[:, j * T:(j + 1) * T],
                            wencv[:, k, mt * 128:(mt + 1) * 128],
                            hsTv[:, k, :], start=(k == 0), stop=(k == KT - 1),
                        )
                zenc_ps.append((ps, mts))

            def zenc_evicts():
                for ps, mts in zenc_ps:
                    for j, mt in enumerate(mts):
                        nc.vector.tensor_scalar_add(
                            zenc[mt][:], ps[:, j * T:(j + 1) * T],
                            benc[:, mt:mt + 1])

            def zdec_block(lo, hi):
                # all 4 mt chunks in one psum tile; evicts trail unchained
                w = hi - lo
                ps = pp.tile([128, VJ], f32, tag="js", bufs=4, name="js")
                for mt in range(KT):
                    for k in range(KT):
                        nc.tensor.matmul(
                            ps[:, mt * w:mt * w + w],
                            wdecv[:, k, mt * 128:(mt + 1) * 128],
                            h1r[:, lo:hi, k], start=(k == 0), stop=(k == KT - 1),
                        )
                for mt in range(KT):
                    nc.vector.tensor_copy(
                        zdec[mt][:, lo:hi], ps[:, mt * w:mt * w + w])

            def lstm_step(u, whv, xgr, hr, c, hist, aux, tg_):
                # streams: PE [48 ifg MMs, 16 o MMs]; the o-tail after the
                # last MM is only go-add -> sigmoid -> h-mul (cell math for
                # c/tanh(c) completes during the o MMs).
                aux_name = "vector" if aux is nc.vector else "gpsimd"
                pifg = pp.tile([128, 12], f32, tag="pifg" + tg_, bufs=1,
                               name="pifg")
                po = pp.tile([128, KT], f32, tag="po" + tg_, bufs=1,
                             name="po")
                gifg = gp.tile([128, 12], f32, tag="gifg" + tg_, name="gifg")
                go = gp.tile([128, KT], f32, tag="go" + tg_, name="go")
                sif = gp.tile([128, 8], f32, tag="sif" + tg_, name="sif")
                tgt = gp.tile([128, KT], f32, tag="tg" + tg_, name="tg")
                so = gp.tile([128, KT], f32, tag="so" + tg_, name="so")
                t1 = gp.tile([128, KT], f32, tag="t1" + tg_, name="t1")
                t2 = gp.tile([128, KT], f32, tag="t2" + tg_, name="t2")
                tc_ = gp.tile([128, KT], f32, tag="tc" + tg_, name="tc")
                if u > 0:
                    for mt in range(12):
                        for k in range(KT):
                            nc.tensor.matmul(
                                pifg[:, mt:mt + 1],
                                whv[:, k, mt * 128:(mt + 1) * 128],
                                hr[:, u - 1, k:k + 1],
                                start=(k == 0), stop=(k == KT - 1),
                            )
                    rec("vector",
                        nc.vector.tensor_add(gifg[:], pifg[:], xgr[:, 0:12, u]))
                else:
                    rec("vector", nc.vector.tensor_copy(gifg[:], xgr[:, 0:12, u]))
                # gate pre-activations are computed at 16x scale (weights
                # and biases staged x16 so W_hh survives fp8e4m3); the
                # activation scale port undoes it exactly.
                rec("scalar", nc.scalar.activation(sif[:], gifg[:, 0:8], Sig,
                                                   scale=1 / 16))
                rec("scalar", nc.scalar.activation(tgt[:], gifg[:, 8:12], Tanh,
                                                   scale=1 / 16))
                if u > 0:
                    for mt in range(12, NG):
                        for k in range(KT):
                            nc.tensor.matmul(
                                po[:, mt - 12:mt - 11],
                                whv[:, k, mt * 128:(mt + 1) * 128],
                                hr[:, u - 1, k:k + 1],
                                start=(k == 0), stop=(k == KT - 1),
                            )
                    rec("vector",
                        nc.vector.tensor_add(go[:], po[:], xgr[:, 12:NG, u]))
                else:
                    rec("vector", nc.vector.tensor_copy(go[:], xgr[:, 12:NG, u]))
                rec("scalar", nc.scalar.activation(so[:], go[:], Sig,
                                                   scale=1 / 16))
                rec(aux_name, aux.tensor_mul(t1[:], sif[:, 4:8], c[:]))
                rec(aux_name, aux.tensor_mul(t2[:], sif[:, 0:4], tgt[:]))
                rec(aux_name, aux.tensor_add(c[:], t1[:], t2[:]))
                rec("scalar", nc.scalar.activation(tc_[:], c[:], Tanh))
                rec(aux_name,
                    aux.tensor_mul(hist[:, u * KT:(u + 1) * KT], so[:], tc_[:]))

            def tanh_u(u):
                for k in range(KT):
                    nc.scalar.activation(
                        tmpT[k][:, u * T:(u + 1) * T], zenc[k][:], Tanh,
                        bias=zdec[k][:, u:u + 1],
                    )

            def joint_rows(m):
                rows = min(128, NT - m * 128)
                ps0 = pp.tile([128, VJ], f32, tag="js", bufs=4, name="js")
                ps1 = pp.tile([128, VJ], f32, tag="js", bufs=4, name="js")
                for k in range(KT):
                    lhs = tmpT[k][:, m * 128:m * 128 + rows]
                    nc.tensor.matmul(ps0[:rows, :], lhs, woutv[:, k, 0:VJ],
                                     start=(k == 0), stop=(k == KT - 1))
                    nc.tensor.matmul(ps1[:rows, :], lhs, woutv[:, k, VJ:ODIM],
                                     start=(k == 0), stop=(k == KT - 1))
                osb = op.tile([128, ODIM], f32, tag="osb", name="osb")
                nc.vector.tensor_add(osb[:rows, 0:VJ], ps0[:rows, :],
                                     bout[:rows, 0:VJ])
                if m == n_m - 1:
                    # split the final tile's store so the first half's DMA
                    # overlaps the second half's eviction (shorter tail)
                    nc.sync.dma_start(d_out[m * 128:m * 128 + rows, 0:VJ],
                                      osb[:rows, 0:VJ])
                    nc.vector.tensor_add(osb[:rows, VJ:ODIM], ps1[:rows, :],
                                         bout[:rows, VJ:ODIM])
                    nc.sync.dma_start(d_out[m * 128:m * 128 + rows, VJ:ODIM],
                                      osb[:rows, VJ:ODIM])
                else:
                    nc.vector.tensor_add(osb[:rows, VJ:ODIM], ps1[:rows, :],
                                         bout[:rows, VJ:ODIM])
                    nc.sync.dma_start(d_out[m * 128:m * 128 + rows, :],
                                      osb[:rows, :])

            # ---- schedule -----------------------------------------------
            n_m = (NT + 127) // 128
            m_done = 0
            u_t = 0           # decoder steps tanh'd
            with scope("RAMP"):
                # A-half (wih0a) work first: the B-half image lands a few
                # us later and must not head-of-line block the PE queue
                xg0_block(0, 8, 0, 8)
                xg0_block(8, U, 0, 4)
                xg0_block(8, U, 4, 8)
                xg0_block(0, 8, 8, NG)
                for t in range(0, LAG):
                    if t < U:
                        lstm_step(t, whh0v, xg0r, h0r, c0, h0h, nc.vector, "0")
                    # fillers sized to fit inside the L0 chain-latency gap
                    if t == 0:
                        xg0_block(8, U, 8, 12)
                    elif t == 1:
                        xg0_block(8, U, 12, NG)
                    elif t == 2:
                        zenc_part((0, 1))
                    elif t == 3:
                        zenc_part((2, 3))
                    if t % XB == XB - 1 and t < U:
                        xg1_block(t - XB + 1, t + 1)

            with scope("LSTM"):
                for t in range(LAG, U):
                    lstm_step(t, whh0v, xg0r, h0r, c0, h0h, nc.vector, "0")
                    if t == 6:
                        zenc_evicts()
                    lstm_step(t - LAG, whh1v, xg1r, h1r, c1, h1h, nc.gpsimd, "1")
                    # block emitted after the L1 step: its eviction lands at
                    # the end of the slot's vector stream, a slot+ before
                    # its first consumer, and never delays the chains.
                    if t % XB == XB - 1:
                        xg1_block(t - XB + 1, t + 1)
                    elif t == U - 1:
                        xg1_block(XB * ((U - 1) // XB), U)

            with scope("DRAIN"):
                # layer-1 tail: interleave zdec blocks, the joint tanh and
                # the first joint row-tiles so the PE stays fed.  The joint
                # ops are left unchained so the scheduler dispatches them
                # around the layer-1 chain by readiness; fillers are emitted
                # BEFORE the L1 step so they run while the PE queue would
                # otherwise wait on the recurrence chain.  PE backlog is
                # primed before the first drain slot (the joint's total PE
                # work dominates the ending, so delaying the L1 tail a bit
                # costs nothing while idle PE slots do).
                zdec_block(0, 16)
                for u in range(8):
                    tanh_u(u)
                u_t = 8
                m_done = 0
                while m_done < (u_t * T) // 128:
                    joint_rows(m_done)
                    m_done += 1
                for i, t in enumerate(range(U, U + LAG)):
                    v = t - LAG                      # L1 step 45..49
                    if i == 0:
                        zdec_block(16, 32)
                    elif i == 1:
                        zdec_block(32, 40)
                    elif i == 4:
                        zdec_block(40, 48)
                    tanh_u(u_t)
                    tanh_u(u_t + 1)
                    tanh_u(u_t + 2)
                    u_t += 3
                    avail = min(((u_t - 1) * T) // 128, n_m)
                    budget = 5
                    while m_done < avail and budget > 0:
                        joint_rows(m_done)
                        m_done += 1
                        budget -= 1
                    lstm_step(v, whh1v, xg1r, h1r, c1, h1h, nc.gpsimd, "1")
                    if v == U - 1:
                        zdec_block(48, U)

            from concourse.tile import add_dep_helper
            for eng, seq in lstm_ord.items():
                for a, b in zip(seq[1:], seq[:-1]):
                    add_dep_helper(a.ins, b.ins, sync=False,
                                   reason="lstm stream order")
            rec_on[0] = False

            sc_j = nc.enter_named_scope("JOINT", False)
            for u in range(u_t, U):
                tanh_u(u)
                avail = ((u + 1) * T) // 128
                while m_done < min(avail, n_m):
                    joint_rows(m_done)
                    m_done += 1
            while m_done < n_m:
                joint_rows(m_done)
                m_done += 1
            nc.leave_named_scope("JOINT", sc_j, False)

    _split_multi_waits(nc)
    return nc


def _stage(inputs):
    f16 = np.float16
    g = lambda k: np.asarray(inputs[k], dtype=np.float32)

    def chunk4(x):
        # [512, W] -> [128, 4*W], chunk k = columns [k*W, (k+1)*W)
        assert x.shape[0] == 4 * 128
        return np.ascontiguousarray(
            np.concatenate([x[k * 128:(k + 1) * 128] for k in range(4)],
                           axis=1))

    hs = g("hs_pad")
    ys = np.asarray(inputs["ys_in_pad"]).astype(np.int64)
    emb0 = g("emb").copy()
    emb0[BLANK] = 0.0
    bg0 = np.ascontiguousarray(
        16.0 * (g("b_ih_0") + g("b_hh_0")).reshape(NG, 128).T)
    bg1 = np.ascontiguousarray(
        16.0 * (g("b_ih_1") + g("b_hh_1")).reshape(NG, 128).T)
    benc = np.ascontiguousarray(g("b_enc").reshape(KT, 128).T)
    bout = np.ascontiguousarray(
        np.broadcast_to(g("b_out"), (128, ODIM)))
    shared = {
        "wih0aT": chunk4((16.0 * g("W_ih_0").T[:, 0:2 * E]).astype(f16)),
        "wih0bT": chunk4((16.0 * g("W_ih_0").T[:, 2 * E:]).astype(f16)),
        "whh0T": chunk4((16.0 * g("W_hh_0").T).astype(f16)),
        "wih1T": chunk4((16.0 * g("W_ih_1").T).astype(f16)),
        "whh1T": chunk4((16.0 * g("W_hh_1").T).astype(f16)),
        "wencT": chunk4(g("W_enc").T.astype(f16)),
        "wdecT": chunk4(g("W_dec").T.astype(f16)),
        "woutT": chunk4(g("W_out").T.astype(f16)),
        "bg0": bg0, "bg1": bg1, "bencT": benc, "boutB": bout,
    }
    in_maps = []
    for b in range(B):
        m = dict(shared)
        m["hsT"] = chunk4(hs[b].T.astype(f16))
        m["eysT"] = chunk4(emb0[ys[b]].T.astype(f16))
        in_maps.append(m)
    return in_maps


def run(inputs, trace=False, ret_res=False):
    from concourse.bass_utils import run_bass_kernel_spmd

    if "nc" not in _CACHE:
        _CACHE["nc"] = _build_nc()
    nc = _CACHE["nc"]
    in_maps = _stage(inputs)
    res = run_bass_kernel_spmd(nc, in_maps, core_ids=list(range(B)),
                               trace=trace)
    _CACHE["last_res"] = res
    out = np.empty((B, T, U, ODIM), np.float32)
    for b in range(B):
        out[b] = res.results[b]["out"].reshape(U, T, ODIM).transpose(1, 0, 2)
    return out, res.exec_time_ns


def kernel(**inputs) -> np.ndarray:
    out, _ = run(inputs, trace=False)
    return out
